# revision 24
# baseline (speedup 1.0000x reference)
"""Trainium2 Bass kernel for HeatmapMaxDetBlock (argmax + local refinement).

Computes, for x[B, C, H, W]:
    scores = max over (H*W); idx = argmax; px = idx % W, py = idx // W (masked
    by score > 0); quarter-pixel refinement by sign of neighbor differences.
Returns [B, C, 3] = (px, py, scores).

Strategy (pure data parallel over 8 NeuronCores, batch-sharded):
  phase 1: stream the whole shard through SBUF once; one DVE reduce_max per
           tile gives per-(row, segment) maxima. Streams at the HBM roofline.
  phase 2: per row group, PE-transpose the maxima, winner-segment select via
           max((M == score) * iorev) (first segment on ties, matching argmax),
           one indirect window gather per group, then fused masked-diff
           refinement (tensor_tensor_reduce / scalar_tensor_tensor):
           rstar = max((win == score) * iorev)     # first peak on ties
           ddx   = sum((iorev == rstar) * (win[+1] - win[-1]))
           ddy   = sum((iorev == rstar) * (win[+W] - win[-W]))
           so no second gather and no find_index pass is needed.
  The 8 leftover rows (group B) stream FIRST and their whole chain hides
  under the main stream (mostly on the otherwise-idle Pool engine); the last
  tiles are tapered (MD 4 -> 2 -> 1) to minimize the exposed final reduce.
"""

import sys
from contextlib import ExitStack
from dataclasses import dataclass

import numpy as np

for _p in ("/opt/trn_rl_repo",):
    if _p not in sys.path:
        sys.path.insert(0, _p)

import concourse.bass as bass  # noqa: E402
import concourse.tile as tile  # noqa: E402
from concourse import bacc, mybir  # noqa: E402
from concourse.masks import make_identity  # noqa: E402

F32 = mybir.dt.float32
U32 = mybir.dt.uint32
I32 = mybir.dt.int32
AX = mybir.AxisListType
OP = mybir.AluOpType


@dataclass(frozen=True)
class Cfg:
    B: int = 64
    C: int = 17
    H: int = 256
    W: int = 192
    ncores: int = 8
    P: int = 128
    NSEG: int = 64
    FRONT: int = 256
    REAR: int = 512

    @property
    def BP(self):  # batches per core
        return self.B // self.ncores

    @property
    def R(self):  # heatmap rows per core
        return self.BP * self.C

    @property
    def HWm(self):
        return self.H * self.W

    @property
    def SEGW(self):
        return self.HWm // self.NSEG

    @property
    def RPT(self):  # rows per tile-column
        return self.P // self.NSEG

    @property
    def NT(self):  # tile-columns per core
        return self.R // self.RPT

    @property
    def MARG(self):
        return self.W + 2

    @property
    def WINW(self):
        return self.SEGW + 2 * self.MARG

    @property
    def SHN(self):
        return self.R * self.HWm

    @property
    def NPAD(self):
        return self.FRONT + self.SHN + self.REAR

    @property
    def NTA(self):  # tile-columns in group A (rows 0..127)
        return self.P // self.RPT

    @property
    def NTB(self):  # tile-columns in group B (rows 128..R-1)
        return self.NT - self.NTA

    @property
    def GB(self):  # rows in group B
        return self.R - self.P


CFG = Cfg()

# stream schedule: group B tile-columns first, then group A with a taper so
# the last reduce is tiny.  Each entry is (start_col, n_cols).
def _schedule(c: Cfg):
    sched = []
    # group B: NTB columns as MD2 chunks
    col = c.NTA
    while col < c.NT:
        md = min(2, c.NT - col)
        sched.append((col, md))
        col += md
    # group A: MD4 until 4 columns remain, then 2, 1, 1
    col = 0
    while col < c.NTA:
        rem = c.NTA - col
        if rem > 4:
            md = 4
        elif rem == 4:
            md = 2
        elif rem >= 2:
            md = min(2, rem - 1) if rem > 1 else 1
        else:
            md = 1
        sched.append((col, md))
        col += md
    return sched


def build_program(cfg: Cfg):
    c = cfg
    assert c.P % c.NSEG == 0 and c.R % c.RPT == 0 and c.HWm % c.NSEG == 0
    assert c.FRONT >= c.MARG and c.REAR >= c.MARG
    sched = _schedule(c)
    assert sum(m for _, m in sched) == c.NT
    assert sorted(q for s, m in sched for q in range(s, s + m)) == list(range(c.NT))

    nc = bacc.Bacc(
        "TRN2", target_bir_lowering=False, debug=False, num_devices=c.ncores
    )
    xh = nc.dram_tensor("x", [c.NPAD], F32, kind="ExternalInput").ap()
    rbh = nc.dram_tensor("rowbase", [c.NT, c.RPT], F32, kind="ExternalInput").ap()
    io64h = nc.dram_tensor("iorev64", [c.NT, c.NSEG], F32, kind="ExternalInput").ap()
    io768h = nc.dram_tensor("iorev768", [c.P, c.SEGW], F32, kind="ExternalInput").ap()
    oh = nc.dram_tensor("out", [c.R, 3], F32, kind="ExternalOutput").ap()

    with ExitStack() as ctx:
        tc = ctx.enter_context(tile.TileContext(nc))
        xpool = ctx.enter_context(tc.tile_pool(name="xp", bufs=6))
        sp = ctx.enter_context(tc.tile_pool(name="sp", bufs=1))
        pp = ctx.enter_context(tc.tile_pool(name="pp", bufs=1, space="PSUM"))

        MA = sp.tile([c.P, c.NTA], F32, tag="MA")
        MB = sp.tile([c.P, c.NTB], F32, tag="MB")

        # constants
        ident = sp.tile([c.P, c.P], F32, tag="ident")
        make_identity(nc, ident[:])
        rbtA = sp.tile([c.NTA, c.RPT], F32, tag="rbtA")
        rbtB = sp.tile([c.NTB, c.RPT], F32, tag="rbtB")
        io64 = sp.tile([c.NT, c.NSEG], F32, tag="io64")
        io768 = sp.tile([c.P, c.SEGW], F32, tag="io768")

        # ---- phase 1: stream + per-(row, segment) maxima ---------------------
        def issue_dma(i, col, md):
            xt = xpool.tile([c.P, md * c.SEGW], F32, tag=f"xt{md}")
            off = c.FRONT + col * c.RPT * c.HWm
            src = bass.AP(
                xh.tensor,
                off,
                [
                    [c.HWm, c.RPT],
                    [c.SEGW, c.NSEG],
                    [c.RPT * c.HWm, md],
                    [1, c.SEGW],
                ],
            )
            eng = nc.sync if i % 2 == 0 else nc.scalar
            eng.dma_start(
                out=xt[:].rearrange("p (m u) -> p m u", m=md), in_=src
            )
            return xt

        def reduce_tile(xt, col, md):
            M, base = (MA, 0) if col < c.NTA else (MB, c.NTA)
            nc.vector.reduce_max(
                out=M[:, col - base : col - base + md],
                in_=xt[:].rearrange("p (m u) -> p m u", m=md),
                axis=AX.X,
            )

        # issue the B DMAs + constants first, then all A DMAs
        nb = c.NTB // 2  # number of B dma chunks
        tiles = {}
        for i, (col, md) in enumerate(sched[:nb]):
            tiles[col] = issue_dma(i, col, md)
        nc.sync.dma_start(out=rbtA[:], in_=rbh[0 : c.NTA])
        nc.sync.dma_start(out=rbtB[:], in_=rbh[c.NTA : c.NT])
        nc.scalar.dma_start(out=io64[:], in_=io64h[:])
        nc.scalar.dma_start(out=io768[:], in_=io768h[:])
        for i, (col, md) in enumerate(sched[nb:]):
            tiles[col] = issue_dma(nb + i, col, md)

        # ---- phase 2 helpers -------------------------------------------------
        def pregather(MT, gp, nt, rbt_s, tagp, cast_eng):
            """winner segment + window start for one group.

            MT: [nt, P] transposed maxima (MT[t, j*NSEG+s]); returns
            (P4 [nt, RPT, 3] packed (w0, score, sb), w0u [gp,1] u32 offsets,
             RT [gp, 3] row-major relayout tile)
            """
            MT3 = MT.rearrange("p (j s) -> p j s", j=c.RPT)
            P4 = sp.tile([nt, c.RPT * 3], F32, tag=f"P4{tagp}")
            P43 = P4[:].rearrange("p (j e) -> p j e", e=3)
            nc.vector.tensor_reduce(
                out=P43[:, :, 1:2], in_=MT3, axis=AX.X, op=OP.max
            )
            # srev = max((M == score) * iorev64) per (t, j); first segment on
            # ties (matches argmax), then sb = (NSEG-1-srev) * SEGW
            srev = sp.tile([nt, c.RPT], F32, tag=f"ss{tagp}")
            for j in range(c.RPT):
                mk = sp.tile([nt, c.NSEG], F32, tag=f"mk{tagp}{j}")
                scr = sp.tile([nt, c.NSEG], F32, tag=f"scr{tagp}{j}")
                nc.vector.tensor_tensor(
                    out=mk[:],
                    in0=MT[:, j * c.NSEG : (j + 1) * c.NSEG],
                    in1=P4[:, 3 * j + 1 : 3 * j + 2].to_broadcast([nt, c.NSEG]),
                    op=OP.is_equal,
                )
                nc.vector.tensor_tensor(
                    out=scr[:], in0=mk[:], in1=io64[0:nt], op=OP.mult
                )
                nc.vector.tensor_reduce(
                    out=srev[:, j : j + 1], in_=scr[:], axis=AX.X, op=OP.max
                )
            nc.vector.tensor_scalar(
                out=P43[:, :, 2:3],
                in0=srev[:, :, None],
                scalar1=-float(c.SEGW),
                scalar2=float((c.NSEG - 1) * c.SEGW),
                op0=OP.mult,
                op1=OP.add,
            )
            nc.vector.tensor_tensor(
                out=P43[:, :, 0:1],
                in0=P43[:, :, 2:3],
                in1=rbt_s[:, :, None],
                op=OP.add,
            )
            RT = sp.tile([gp, 3], F32, tag=f"RT{tagp}")
            # group B's relayout must NOT queue behind the stream DMAs on
            # sync/scalar (their issue paces with the reduces) — use the
            # otherwise-idle gpsimd SWDGE queue for it.
            dma_eng = nc.sync if tagp == "a" else nc.scalar
            dma_eng.dma_start(out=RT[:], in_=P43)
            w0u = sp.tile([gp, 1], U32, tag=f"w0u{tagp}")
            cast_eng.tensor_copy(out=w0u[:], in_=RT[:, 0:1])
            return RT, w0u

        def gather_win(gp, w0u, tagp):
            win = sp.tile([gp, c.WINW], F32, tag=f"win{tagp}")
            nc.gpsimd.indirect_dma_start(
                out=win[:],
                out_offset=None,
                in_=xh[:, None],
                in_offset=bass.IndirectOffsetOnAxis(ap=w0u[:, 0:1], axis=0),
            )
            return win

        def postgather(win, RT, gp, tagp, div_eng, aux_eng):
            """masked-diff refinement + coordinate math for one group."""
            M0 = c.MARG
            mid = win[:, M0 : M0 + c.SEGW]
            diff = sp.tile([gp, 2 * c.SEGW], F32, tag=f"df{tagp}")
            aux_eng.tensor_tensor(
                out=diff[:, 0 : c.SEGW],
                in0=win[:, M0 + 1 : M0 + 1 + c.SEGW],
                in1=win[:, M0 - 1 : M0 - 1 + c.SEGW],
                op=OP.subtract,
            )
            aux_eng.tensor_tensor(
                out=diff[:, c.SEGW : 2 * c.SEGW],
                in0=win[:, M0 + c.W : M0 + c.W + c.SEGW],
                in1=win[:, M0 - c.W : M0 - c.W + c.SEGW],
                op=OP.subtract,
            )
            scr = sp.tile([gp, c.SEGW], F32, tag=f"pscr{tagp}")
            mkw = sp.tile([gp, c.SEGW], F32, tag=f"mkw{tagp}")
            ii = sp.tile([gp, 1], F32, tag=f"ii{tagp}")
            rstar = sp.tile([gp, 1], F32, tag=f"rs{tagp}")
            D = sp.tile([gp, 2], F32, tag=f"D{tagp}")
            score = RT[:, 1:2]
            # rstar = max((win == score) * iorev): first peak position on ties
            nc.vector.tensor_tensor(
                out=mkw[:], in0=mid,
                in1=score.to_broadcast([gp, c.SEGW]), op=OP.is_equal,
            )
            nc.vector.tensor_tensor(
                out=scr[:], in0=mkw[:], in1=io768[0:gp], op=OP.mult
            )
            nc.vector.tensor_reduce(
                out=rstar[:], in_=scr[:], axis=AX.X, op=OP.max
            )
            # one-hot select the +-1 / +-W differences at that exact position
            nc.vector.scalar_tensor_tensor(
                out=scr[:], in0=io768[0:gp], scalar=rstar[:],
                in1=diff[:, 0 : c.SEGW],
                op0=OP.is_equal, op1=OP.mult, accum_out=D[:, 0:1],
            )
            nc.vector.scalar_tensor_tensor(
                out=scr[:], in0=io768[0:gp], scalar=rstar[:],
                in1=diff[:, c.SEGW :],
                op0=OP.is_equal, op1=OP.mult, accum_out=D[:, 1:2],
            )

            # flat index within row; px, py via exact f32 division fixup
            O = sp.tile([gp, 3], F32, tag=f"O{tagp}")
            e = div_eng
            e.tensor_scalar(
                out=ii[:], in0=rstar[:], scalar1=-1.0,
                scalar2=float(c.SEGW - 1), op0=OP.mult, op1=OP.add,
            )
            idxm = sp.tile([gp, 1], F32, tag=f"idxm{tagp}")
            e.tensor_tensor(out=idxm[:], in0=RT[:, 2:3], in1=ii[:], op=OP.add)
            t1 = sp.tile([gp, 1], F32, tag=f"t1{tagp}")
            t2 = sp.tile([gp, 1], F32, tag=f"t2{tagp}")
            qi = sp.tile([gp, 1], I32, tag=f"qi{tagp}")
            e.tensor_scalar(
                out=t1[:], in0=idxm[:], scalar1=1.0 / c.W, scalar2=0.0013,
                op0=OP.mult, op1=OP.add,
            )
            e.tensor_copy(out=qi[:], in_=t1[:])
            e.tensor_copy(out=t1[:], in_=qi[:])
            e.tensor_scalar(
                out=t2[:], in0=t1[:], scalar1=-float(c.W), scalar2=None,
                op0=OP.mult,
            )
            e.tensor_tensor(out=t2[:], in0=idxm[:], in1=t2[:], op=OP.add)
            lo = sp.tile([gp, 1], F32, tag=f"lo{tagp}")
            e.tensor_scalar(
                out=lo[:], in0=t2[:], scalar1=0.0, scalar2=None, op0=OP.is_lt
            )
            e.tensor_tensor(out=t1[:], in0=t1[:], in1=lo[:], op=OP.subtract)
            e.tensor_scalar(
                out=lo[:], in0=t2[:], scalar1=float(c.W), scalar2=None,
                op0=OP.is_ge,
            )
            e.tensor_tensor(out=O[:, 1:2], in0=t1[:], in1=lo[:], op=OP.add)
            e.tensor_scalar(
                out=t2[:], in0=O[:, 1:2], scalar1=-float(c.W), scalar2=None,
                op0=OP.mult,
            )
            e.tensor_tensor(out=O[:, 0:1], in0=idxm[:], in1=t2[:], op=OP.add)
            mk = sp.tile([gp, 1], F32, tag=f"mk{tagp}")
            e.tensor_scalar(
                out=mk[:], in0=score, scalar1=0.0, scalar2=None, op0=OP.is_gt
            )
            e.tensor_tensor(
                out=O[:, 0:2], in0=O[:, 0:2],
                in1=mk[:].to_broadcast([gp, 2]), op=OP.mult,
            )
            # interior = (0 < px < W-1) & (0 < py < H-1)
            hi = sp.tile([gp, 2], F32, tag=f"hi{tagp}")
            nc.vector.memset(hi[:, 0:1], float(c.W - 1))
            nc.vector.memset(hi[:, 1:2], float(c.H - 1))
            ilo = sp.tile([gp, 2], F32, tag=f"ilo{tagp}")
            e.tensor_scalar(
                out=ilo[:], in0=O[:, 0:2], scalar1=0.0, scalar2=None,
                op0=OP.is_gt,
            )
            ihi = sp.tile([gp, 2], F32, tag=f"ihi{tagp}")
            e.tensor_tensor(out=ihi[:], in0=O[:, 0:2], in1=hi[:], op=OP.is_lt)
            e.tensor_tensor(out=ilo[:], in0=ilo[:], in1=ihi[:], op=OP.mult)
            intr = sp.tile([gp, 1], F32, tag=f"intr{tagp}")
            nc.vector.tensor_reduce(
                out=intr[:], in_=ilo[:], axis=AX.X, op=OP.min
            )
            # dx, dy = 0.25 * sign(D) * interior
            DG = sp.tile([gp, 2], F32, tag=f"DG{tagp}")
            DL = sp.tile([gp, 2], F32, tag=f"DL{tagp}")
            e.tensor_scalar(
                out=DG[:], in0=D[:], scalar1=0.0, scalar2=0.25,
                op0=OP.is_gt, op1=OP.mult,
            )
            e.tensor_scalar(
                out=DL[:], in0=D[:], scalar1=0.0, scalar2=0.25,
                op0=OP.is_lt, op1=OP.mult,
            )
            e.tensor_tensor(out=DG[:], in0=DG[:], in1=DL[:], op=OP.subtract)
            e.tensor_tensor(
                out=DG[:], in0=DG[:], in1=intr[:].to_broadcast([gp, 2]),
                op=OP.mult,
            )
            e.tensor_tensor(out=O[:, 0:2], in0=O[:, 0:2], in1=DG[:], op=OP.add)
            e.tensor_copy(out=O[:, 2:3], in_=score)
            return O

        # ---- group B (rows P..R-1): reduce, chain hidden under the stream ----
        for col, md in sched[:nb]:
            reduce_tile(tiles[col], col, md)
        mtpB = pp.tile([c.NTB, c.P], F32, tag="mtpB")
        nc.tensor.transpose(out=mtpB[:], in_=MB[:], identity=ident[:])
        MTB = sp.tile([c.NTB, c.P], F32, tag="MTB")
        nc.vector.tensor_copy(out=MTB[:], in_=mtpB[:])
        RTB, w0uB = pregather(
            MTB[:], c.GB, c.NTB, rbtB[:], "b", nc.vector
        )
        winB = gather_win(c.GB, w0uB, "b")

        # ---- group A reduces, with B's post-gather interleaved early ---------
        na = len(sched) - nb
        for k, (col, md) in enumerate(sched[nb:]):
            reduce_tile(tiles[col], col, md)
            if k == 3:
                OB = postgather(winB[:], RTB[:], c.GB, "b", nc.vector, nc.gpsimd)
                nc.scalar.dma_start(out=oh[c.P : c.R], in_=OB[:])

        # ---- group A chain ---------------------------------------------------
        mtpA = pp.tile([c.NTA, c.P], F32, tag="mtpA")
        nc.tensor.transpose(out=mtpA[:], in_=MA[:], identity=ident[:])
        MTA = sp.tile([c.NTA, c.P], F32, tag="MTA")
        nc.vector.tensor_copy(out=MTA[:], in_=mtpA[:])
        RTA, w0uA = pregather(
            MTA[:], c.P, c.NTA, rbtA[:], "a", nc.vector
        )
        winA = gather_win(c.P, w0uA, "a")
        OA = postgather(winA[:], RTA[:], c.P, "a", nc.vector, nc.gpsimd)
        nc.sync.dma_start(out=oh[0 : c.P], in_=OA[:])

    nc.compile()
    return nc


def host_constants(cfg: Cfg):
    c = cfg
    r = np.arange(c.R, dtype=np.float64)
    rowbase = (c.FRONT + r * c.HWm - c.MARG).astype(np.float32).reshape(c.NT, c.RPT)
    iorev64 = np.tile(
        (c.NSEG - 1 - np.arange(c.NSEG)).astype(np.float32), (c.NT, 1)
    )
    iorev768 = np.tile(
        (c.SEGW - 1 - np.arange(c.SEGW)).astype(np.float32), (c.P, 1)
    )
    return rowbase, iorev64, iorev768


def shard_inputs(cfg: Cfg, x: np.ndarray):
    c = cfg
    rowbase, iorev64, iorev768 = host_constants(c)
    in_maps = []
    for k in range(c.ncores):
        shard = np.ascontiguousarray(
            x[k * c.BP : (k + 1) * c.BP], dtype=np.float32
        ).reshape(-1)
        xp = np.zeros(c.NPAD, np.float32)
        xp[c.FRONT : c.FRONT + c.SHN] = shard
        in_maps.append(
            {"x": xp, "rowbase": rowbase, "iorev64": iorev64, "iorev768": iorev768}
        )
    return in_maps


def assemble_out(cfg: Cfg, per_core_outs):
    c = cfg
    outs = [o.reshape(c.BP, c.C, 3).astype(np.float32) for o in per_core_outs]
    return np.concatenate(outs, axis=0)


_PROGRAM = None


def _program():
    global _PROGRAM
    if _PROGRAM is None:
        _PROGRAM = build_program(CFG)
    return _PROGRAM


def kernel(x: np.ndarray) -> np.ndarray:
    from concourse.bass_utils import run_bass_kernel_spmd

    c = CFG
    assert x.shape == (c.B, c.C, c.H, c.W), x.shape
    nc = _program()
    in_maps = shard_inputs(c, np.asarray(x))
    res = run_bass_kernel_spmd(nc, in_maps, core_ids=list(range(c.ncores)))
    return assemble_out(c, [res.results[k]["out"] for k in range(c.ncores)])


# revision 25
# speedup vs baseline: 1.0461x; 1.0461x over previous
"""Trainium2 Bass kernel for HeatmapMaxDetBlock (argmax + local refinement).

Computes, for x[B, C, H, W]:
    scores = max over (H*W); idx = argmax; px = idx % W, py = idx // W (masked
    by score > 0); quarter-pixel refinement by sign of neighbor differences.
Returns [B, C, 3] = (px, py, scores).

Strategy (pure data parallel over 8 NeuronCores, batch-sharded):
  phase 1: stream the whole shard through SBUF once; one DVE reduce_max per
           tile gives per-(row, segment) maxima. Streams at the HBM roofline.
  phase 2: per row group, PE-transpose the maxima, winner-segment select via
           max((M == score) * iorev) (first segment on ties, matching argmax),
           one indirect window gather per group, then fused masked-diff
           refinement (tensor_tensor_reduce / scalar_tensor_tensor):
           rstar = max((win == score) * iorev)     # first peak on ties
           ddx   = sum((iorev == rstar) * (win[+1] - win[-1]))
           ddy   = sum((iorev == rstar) * (win[+W] - win[-W]))
           so no second gather and no find_index pass is needed.
  The 8 leftover rows (group B) stream FIRST and their whole chain hides
  under the main stream (mostly on the otherwise-idle Pool engine); the last
  tiles are tapered (MD 4 -> 2 -> 1) to minimize the exposed final reduce.
"""

import sys
from contextlib import ExitStack
from dataclasses import dataclass

import numpy as np

for _p in ("/opt/trn_rl_repo",):
    if _p not in sys.path:
        sys.path.insert(0, _p)

import concourse.bass as bass  # noqa: E402
import concourse.tile as tile  # noqa: E402
from concourse import bacc, mybir  # noqa: E402
from concourse.masks import make_identity  # noqa: E402

F32 = mybir.dt.float32
U32 = mybir.dt.uint32
I32 = mybir.dt.int32
AX = mybir.AxisListType
OP = mybir.AluOpType


@dataclass(frozen=True)
class Cfg:
    B: int = 64
    C: int = 17
    H: int = 256
    W: int = 192
    ncores: int = 8
    P: int = 128
    NSEG: int = 64
    FRONT: int = 256
    REAR: int = 512

    @property
    def BP(self):  # batches per core
        return self.B // self.ncores

    @property
    def R(self):  # heatmap rows per core
        return self.BP * self.C

    @property
    def HWm(self):
        return self.H * self.W

    @property
    def SEGW(self):
        return self.HWm // self.NSEG

    @property
    def RPT(self):  # rows per tile-column
        return self.P // self.NSEG

    @property
    def NT(self):  # tile-columns per core
        return self.R // self.RPT

    @property
    def MARG(self):
        return self.W + 2

    @property
    def WINW(self):
        return self.SEGW + 2 * self.MARG

    @property
    def SHN(self):
        return self.R * self.HWm

    @property
    def NPAD(self):
        return self.FRONT + self.SHN + self.REAR

    @property
    def NTA(self):  # tile-columns in group A (rows 0..127)
        return self.P // self.RPT

    @property
    def NTB(self):  # tile-columns in group B (rows 128..R-1)
        return self.NT - self.NTA

    @property
    def GB(self):  # rows in group B
        return self.R - self.P


CFG = Cfg()

# stream schedule: group B tile-columns first, then group A with a taper so
# the last reduce is tiny.  Each entry is (start_col, n_cols).
def _schedule(c: Cfg):
    sched = []
    # group B: NTB columns as MD2 chunks
    col = c.NTA
    while col < c.NT:
        md = min(2, c.NT - col)
        sched.append((col, md))
        col += md
    # group A: MD4 until 4 columns remain, then 2, 1, 1
    col = 0
    while col < c.NTA:
        rem = c.NTA - col
        if rem > 4:
            md = 4
        elif rem == 4:
            md = 2
        elif rem >= 2:
            md = min(2, rem - 1) if rem > 1 else 1
        else:
            md = 1
        sched.append((col, md))
        col += md
    return sched


def build_program(cfg: Cfg):
    c = cfg
    assert c.P % c.NSEG == 0 and c.R % c.RPT == 0 and c.HWm % c.NSEG == 0
    assert c.FRONT >= c.MARG and c.REAR >= c.MARG
    sched = _schedule(c)
    assert sum(m for _, m in sched) == c.NT
    assert sorted(q for s, m in sched for q in range(s, s + m)) == list(range(c.NT))

    nc = bacc.Bacc(
        "TRN2", target_bir_lowering=False, debug=False, num_devices=c.ncores
    )
    xh = nc.dram_tensor("x", [c.NPAD], F32, kind="ExternalInput").ap()
    rbh = nc.dram_tensor("rowbase", [c.NT, c.RPT], F32, kind="ExternalInput").ap()
    io64h = nc.dram_tensor("iorev64", [c.NT, c.NSEG], F32, kind="ExternalInput").ap()
    io768h = nc.dram_tensor("iorev768", [c.P, c.SEGW], F32, kind="ExternalInput").ap()
    oh = nc.dram_tensor("out", [c.R, 3], F32, kind="ExternalOutput").ap()

    with ExitStack() as ctx:
        tc = ctx.enter_context(tile.TileContext(nc))
        xpool = ctx.enter_context(tc.tile_pool(name="xp", bufs=6))
        sp = ctx.enter_context(tc.tile_pool(name="sp", bufs=1))
        pp = ctx.enter_context(tc.tile_pool(name="pp", bufs=1, space="PSUM"))

        MA = sp.tile([c.P, c.NTA], F32, tag="MA")
        MB = sp.tile([c.P, c.NTB], F32, tag="MB")

        # constants
        ident = sp.tile([c.P, c.P], F32, tag="ident")
        make_identity(nc, ident[:])
        rbtA = sp.tile([c.NTA, c.RPT], F32, tag="rbtA")
        rbtB = sp.tile([c.NTB, c.RPT], F32, tag="rbtB")
        io64 = sp.tile([c.NT, c.NSEG], F32, tag="io64")
        io768 = sp.tile([c.P, c.SEGW], F32, tag="io768")

        # ---- phase 1: stream + per-(row, segment) maxima ---------------------
        def issue_dma(i, col, md):
            xt = xpool.tile([c.P, md * c.SEGW], F32, tag=f"xt{md}")
            off = c.FRONT + col * c.RPT * c.HWm
            src = bass.AP(
                xh.tensor,
                off,
                [
                    [c.HWm, c.RPT],
                    [c.SEGW, c.NSEG],
                    [c.RPT * c.HWm, md],
                    [1, c.SEGW],
                ],
            )
            eng = nc.sync if i % 2 == 0 else nc.scalar
            eng.dma_start(
                out=xt[:].rearrange("p (m u) -> p m u", m=md), in_=src
            )
            return xt

        def reduce_tile(xt, col, md):
            M, base = (MA, 0) if col < c.NTA else (MB, c.NTA)
            nc.vector.reduce_max(
                out=M[:, col - base : col - base + md],
                in_=xt[:].rearrange("p (m u) -> p m u", m=md),
                axis=AX.X,
            )

        # issue the B DMAs + constants first, then all A DMAs
        nb = c.NTB // 2  # number of B dma chunks
        tiles = {}
        for i, (col, md) in enumerate(sched[:nb]):
            tiles[col] = issue_dma(i, col, md)
        nc.sync.dma_start(out=rbtA[:], in_=rbh[0 : c.NTA])
        nc.sync.dma_start(out=rbtB[:], in_=rbh[c.NTA : c.NT])
        nc.scalar.dma_start(out=io64[:], in_=io64h[:])
        nc.scalar.dma_start(out=io768[:], in_=io768h[:])
        for i, (col, md) in enumerate(sched[nb:]):
            tiles[col] = issue_dma(nb + i, col, md)

        # ---- phase 2 helpers -------------------------------------------------
        def pregather(MT, gp, nt, rbt_s, tagp, cast_eng):
            """winner segment + window start for one group.

            MT: [nt, P] transposed maxima (MT[t, j*NSEG+s]); returns
            (P4 [nt, RPT, 3] packed (w0, score, sb), w0u [gp,1] u32 offsets,
             RT [gp, 3] row-major relayout tile)
            """
            MT3 = MT.rearrange("p (j s) -> p j s", j=c.RPT)
            P4 = sp.tile([nt, c.RPT * 3], F32, tag=f"P4{tagp}")
            P43 = P4[:].rearrange("p (j e) -> p j e", e=3)
            nc.vector.tensor_reduce(
                out=P43[:, :, 1:2], in_=MT3, axis=AX.X, op=OP.max
            )
            # srev = max((M == score) * iorev64) per (t, j); first segment on
            # ties (matches argmax), then sb = (NSEG-1-srev) * SEGW
            srev = sp.tile([nt, c.RPT], F32, tag=f"ss{tagp}")
            for j in range(c.RPT):
                mk = sp.tile([nt, c.NSEG], F32, tag=f"mk{tagp}{j}")
                scr = sp.tile([nt, c.NSEG], F32, tag=f"scr{tagp}{j}")
                nc.vector.tensor_tensor(
                    out=mk[:],
                    in0=MT[:, j * c.NSEG : (j + 1) * c.NSEG],
                    in1=P4[:, 3 * j + 1 : 3 * j + 2].to_broadcast([nt, c.NSEG]),
                    op=OP.is_equal,
                )
                nc.vector.tensor_tensor(
                    out=scr[:], in0=mk[:], in1=io64[0:nt], op=OP.mult
                )
                nc.vector.tensor_reduce(
                    out=srev[:, j : j + 1], in_=scr[:], axis=AX.X, op=OP.max
                )
            nc.vector.tensor_scalar(
                out=P43[:, :, 2:3],
                in0=srev[:, :, None],
                scalar1=-float(c.SEGW),
                scalar2=float((c.NSEG - 1) * c.SEGW),
                op0=OP.mult,
                op1=OP.add,
            )
            nc.vector.tensor_tensor(
                out=P43[:, :, 0:1],
                in0=P43[:, :, 2:3],
                in1=rbt_s[:, :, None],
                op=OP.add,
            )
            RT = sp.tile([gp, 3], F32, tag=f"RT{tagp}")
            # group B's relayout must NOT queue behind the stream DMAs on
            # sync/scalar (their issue paces with the reduces) — use the
            # otherwise-idle gpsimd SWDGE queue for it.
            dma_eng = nc.sync if tagp == "a" else nc.gpsimd
            dma_eng.dma_start(out=RT[:], in_=P43)
            w0u = sp.tile([gp, 1], U32, tag=f"w0u{tagp}")
            cast_eng.tensor_copy(out=w0u[:], in_=RT[:, 0:1])
            return RT, w0u

        def gather_win(gp, w0u, tagp):
            win = sp.tile([gp, c.WINW], F32, tag=f"win{tagp}")
            nc.gpsimd.indirect_dma_start(
                out=win[:],
                out_offset=None,
                in_=xh[:, None],
                in_offset=bass.IndirectOffsetOnAxis(ap=w0u[:, 0:1], axis=0),
            )
            return win

        def postgather(win, RT, gp, tagp, div_eng, aux_eng):
            """masked-diff refinement + coordinate math for one group."""
            M0 = c.MARG
            mid = win[:, M0 : M0 + c.SEGW]
            diff = sp.tile([gp, 2 * c.SEGW], F32, tag=f"df{tagp}")
            aux_eng.tensor_tensor(
                out=diff[:, 0 : c.SEGW],
                in0=win[:, M0 + 1 : M0 + 1 + c.SEGW],
                in1=win[:, M0 - 1 : M0 - 1 + c.SEGW],
                op=OP.subtract,
            )
            aux_eng.tensor_tensor(
                out=diff[:, c.SEGW : 2 * c.SEGW],
                in0=win[:, M0 + c.W : M0 + c.W + c.SEGW],
                in1=win[:, M0 - c.W : M0 - c.W + c.SEGW],
                op=OP.subtract,
            )
            scr = sp.tile([gp, c.SEGW], F32, tag=f"pscr{tagp}")
            mkw = sp.tile([gp, c.SEGW], F32, tag=f"mkw{tagp}")
            ii = sp.tile([gp, 1], F32, tag=f"ii{tagp}")
            rstar = sp.tile([gp, 1], F32, tag=f"rs{tagp}")
            D = sp.tile([gp, 2], F32, tag=f"D{tagp}")
            score = RT[:, 1:2]
            # rstar = max((win == score) * iorev): first peak position on ties
            nc.vector.tensor_tensor(
                out=mkw[:], in0=mid,
                in1=score.to_broadcast([gp, c.SEGW]), op=OP.is_equal,
            )
            nc.vector.tensor_tensor(
                out=scr[:], in0=mkw[:], in1=io768[0:gp], op=OP.mult
            )
            nc.vector.tensor_reduce(
                out=rstar[:], in_=scr[:], axis=AX.X, op=OP.max
            )
            # one-hot select the +-1 / +-W differences at that exact position
            nc.vector.scalar_tensor_tensor(
                out=scr[:], in0=io768[0:gp], scalar=rstar[:],
                in1=diff[:, 0 : c.SEGW],
                op0=OP.is_equal, op1=OP.mult, accum_out=D[:, 0:1],
            )
            nc.vector.scalar_tensor_tensor(
                out=scr[:], in0=io768[0:gp], scalar=rstar[:],
                in1=diff[:, c.SEGW :],
                op0=OP.is_equal, op1=OP.mult, accum_out=D[:, 1:2],
            )

            # flat index within row; px, py via exact f32 division fixup
            O = sp.tile([gp, 3], F32, tag=f"O{tagp}")
            e = div_eng
            e.tensor_scalar(
                out=ii[:], in0=rstar[:], scalar1=-1.0,
                scalar2=float(c.SEGW - 1), op0=OP.mult, op1=OP.add,
            )
            idxm = sp.tile([gp, 1], F32, tag=f"idxm{tagp}")
            e.tensor_tensor(out=idxm[:], in0=RT[:, 2:3], in1=ii[:], op=OP.add)
            t1 = sp.tile([gp, 1], F32, tag=f"t1{tagp}")
            t2 = sp.tile([gp, 1], F32, tag=f"t2{tagp}")
            qi = sp.tile([gp, 1], I32, tag=f"qi{tagp}")
            e.tensor_scalar(
                out=t1[:], in0=idxm[:], scalar1=1.0 / c.W, scalar2=0.0013,
                op0=OP.mult, op1=OP.add,
            )
            e.tensor_copy(out=qi[:], in_=t1[:])
            e.tensor_copy(out=t1[:], in_=qi[:])
            e.tensor_scalar(
                out=t2[:], in0=t1[:], scalar1=-float(c.W), scalar2=None,
                op0=OP.mult,
            )
            e.tensor_tensor(out=t2[:], in0=idxm[:], in1=t2[:], op=OP.add)
            lo = sp.tile([gp, 1], F32, tag=f"lo{tagp}")
            e.tensor_scalar(
                out=lo[:], in0=t2[:], scalar1=0.0, scalar2=None, op0=OP.is_lt
            )
            e.tensor_tensor(out=t1[:], in0=t1[:], in1=lo[:], op=OP.subtract)
            e.tensor_scalar(
                out=lo[:], in0=t2[:], scalar1=float(c.W), scalar2=None,
                op0=OP.is_ge,
            )
            e.tensor_tensor(out=O[:, 1:2], in0=t1[:], in1=lo[:], op=OP.add)
            e.tensor_scalar(
                out=t2[:], in0=O[:, 1:2], scalar1=-float(c.W), scalar2=None,
                op0=OP.mult,
            )
            e.tensor_tensor(out=O[:, 0:1], in0=idxm[:], in1=t2[:], op=OP.add)
            mk = sp.tile([gp, 1], F32, tag=f"mk{tagp}")
            e.tensor_scalar(
                out=mk[:], in0=score, scalar1=0.0, scalar2=None, op0=OP.is_gt
            )
            e.tensor_tensor(
                out=O[:, 0:2], in0=O[:, 0:2],
                in1=mk[:].to_broadcast([gp, 2]), op=OP.mult,
            )
            # interior = (0 < px < W-1) & (0 < py < H-1)
            hi = sp.tile([gp, 2], F32, tag=f"hi{tagp}")
            nc.vector.memset(hi[:, 0:1], float(c.W - 1))
            nc.vector.memset(hi[:, 1:2], float(c.H - 1))
            ilo = sp.tile([gp, 2], F32, tag=f"ilo{tagp}")
            e.tensor_scalar(
                out=ilo[:], in0=O[:, 0:2], scalar1=0.0, scalar2=None,
                op0=OP.is_gt,
            )
            ihi = sp.tile([gp, 2], F32, tag=f"ihi{tagp}")
            e.tensor_tensor(out=ihi[:], in0=O[:, 0:2], in1=hi[:], op=OP.is_lt)
            e.tensor_tensor(out=ilo[:], in0=ilo[:], in1=ihi[:], op=OP.mult)
            intr = sp.tile([gp, 1], F32, tag=f"intr{tagp}")
            nc.vector.tensor_reduce(
                out=intr[:], in_=ilo[:], axis=AX.X, op=OP.min
            )
            # dx, dy = 0.25 * sign(D) * interior
            DG = sp.tile([gp, 2], F32, tag=f"DG{tagp}")
            DL = sp.tile([gp, 2], F32, tag=f"DL{tagp}")
            e.tensor_scalar(
                out=DG[:], in0=D[:], scalar1=0.0, scalar2=0.25,
                op0=OP.is_gt, op1=OP.mult,
            )
            e.tensor_scalar(
                out=DL[:], in0=D[:], scalar1=0.0, scalar2=0.25,
                op0=OP.is_lt, op1=OP.mult,
            )
            e.tensor_tensor(out=DG[:], in0=DG[:], in1=DL[:], op=OP.subtract)
            e.tensor_tensor(
                out=DG[:], in0=DG[:], in1=intr[:].to_broadcast([gp, 2]),
                op=OP.mult,
            )
            e.tensor_tensor(out=O[:, 0:2], in0=O[:, 0:2], in1=DG[:], op=OP.add)
            e.tensor_copy(out=O[:, 2:3], in_=score)
            return O

        # ---- group B (rows P..R-1): reduce, chain hidden under the stream ----
        for col, md in sched[:nb]:
            reduce_tile(tiles[col], col, md)
        mtpB = pp.tile([c.NTB, c.P], F32, tag="mtpB")
        nc.tensor.transpose(out=mtpB[:], in_=MB[:], identity=ident[:])
        MTB = sp.tile([c.NTB, c.P], F32, tag="MTB")
        nc.vector.tensor_copy(out=MTB[:], in_=mtpB[:])
        RTB, w0uB = pregather(
            MTB[:], c.GB, c.NTB, rbtB[:], "b", nc.vector
        )
        winB = gather_win(c.GB, w0uB, "b")

        # ---- group A reduces, with B's post-gather interleaved early ---------
        na = len(sched) - nb
        for k, (col, md) in enumerate(sched[nb:]):
            reduce_tile(tiles[col], col, md)
            if k == 3:
                OB = postgather(winB[:], RTB[:], c.GB, "b", nc.vector, nc.gpsimd)
                nc.gpsimd.dma_start(out=oh[c.P : c.R], in_=OB[:])

        # ---- group A chain ---------------------------------------------------
        mtpA = pp.tile([c.NTA, c.P], F32, tag="mtpA")
        nc.tensor.transpose(out=mtpA[:], in_=MA[:], identity=ident[:])
        MTA = sp.tile([c.NTA, c.P], F32, tag="MTA")
        nc.vector.tensor_copy(out=MTA[:], in_=mtpA[:])
        RTA, w0uA = pregather(
            MTA[:], c.P, c.NTA, rbtA[:], "a", nc.vector
        )
        winA = gather_win(c.P, w0uA, "a")
        OA = postgather(winA[:], RTA[:], c.P, "a", nc.vector, nc.gpsimd)
        nc.sync.dma_start(out=oh[0 : c.P], in_=OA[:])

    nc.compile()
    return nc


def host_constants(cfg: Cfg):
    c = cfg
    r = np.arange(c.R, dtype=np.float64)
    rowbase = (c.FRONT + r * c.HWm - c.MARG).astype(np.float32).reshape(c.NT, c.RPT)
    iorev64 = np.tile(
        (c.NSEG - 1 - np.arange(c.NSEG)).astype(np.float32), (c.NT, 1)
    )
    iorev768 = np.tile(
        (c.SEGW - 1 - np.arange(c.SEGW)).astype(np.float32), (c.P, 1)
    )
    return rowbase, iorev64, iorev768


def shard_inputs(cfg: Cfg, x: np.ndarray):
    c = cfg
    rowbase, iorev64, iorev768 = host_constants(c)
    in_maps = []
    for k in range(c.ncores):
        shard = np.ascontiguousarray(
            x[k * c.BP : (k + 1) * c.BP], dtype=np.float32
        ).reshape(-1)
        xp = np.zeros(c.NPAD, np.float32)
        xp[c.FRONT : c.FRONT + c.SHN] = shard
        in_maps.append(
            {"x": xp, "rowbase": rowbase, "iorev64": iorev64, "iorev768": iorev768}
        )
    return in_maps


def assemble_out(cfg: Cfg, per_core_outs):
    c = cfg
    outs = [o.reshape(c.BP, c.C, 3).astype(np.float32) for o in per_core_outs]
    return np.concatenate(outs, axis=0)


_PROGRAM = None


def _program():
    global _PROGRAM
    if _PROGRAM is None:
        _PROGRAM = build_program(CFG)
    return _PROGRAM


def kernel(x: np.ndarray) -> np.ndarray:
    from concourse.bass_utils import run_bass_kernel_spmd

    c = CFG
    assert x.shape == (c.B, c.C, c.H, c.W), x.shape
    nc = _program()
    in_maps = shard_inputs(c, np.asarray(x))
    res = run_bass_kernel_spmd(nc, in_maps, core_ids=list(range(c.ncores)))
    return assemble_out(c, [res.results[k]["out"] for k in range(c.ncores)])


# revision 28
# speedup vs baseline: 1.0872x; 1.0393x over previous
"""Trainium2 Bass kernel for HeatmapMaxDetBlock (argmax + local refinement).

Computes, for x[B, C, H, W]:
    scores = max over (H*W); idx = argmax; px = idx % W, py = idx // W (masked
    by score > 0); quarter-pixel refinement by sign of neighbor differences.
Returns [B, C, 3] = (px, py, scores).

Strategy (pure data parallel over 8 NeuronCores, batch-sharded):
  phase 1: stream the whole shard through SBUF once; one DVE reduce_max per
           tile gives per-(row, segment) maxima. Streams at the HBM roofline.
  phase 2: per row group, PE-transpose the maxima, winner-segment select via
           max((M == score) * iorev) (first segment on ties, matching argmax),
           one indirect window gather per group, then fused masked-diff
           refinement (tensor_tensor_reduce / scalar_tensor_tensor):
           rstar = max((win == score) * iorev)     # first peak on ties
           ddx   = sum((iorev == rstar) * (win[+1] - win[-1]))
           ddy   = sum((iorev == rstar) * (win[+W] - win[-W]))
           so no second gather and no find_index pass is needed.
  The 8 leftover rows (group B) stream FIRST and their whole chain hides
  under the main stream (mostly on the otherwise-idle Pool engine); the last
  tiles are tapered (MD 4 -> 2 -> 1) to minimize the exposed final reduce.
"""

import sys
from contextlib import ExitStack
from dataclasses import dataclass

import numpy as np

for _p in ("/opt/trn_rl_repo",):
    if _p not in sys.path:
        sys.path.insert(0, _p)

import concourse.bass as bass  # noqa: E402
import concourse.tile as tile  # noqa: E402
from concourse import bacc, mybir  # noqa: E402
from concourse.masks import make_identity  # noqa: E402

F32 = mybir.dt.float32
U32 = mybir.dt.uint32
I32 = mybir.dt.int32
AX = mybir.AxisListType
OP = mybir.AluOpType


@dataclass(frozen=True)
class Cfg:
    B: int = 64
    C: int = 17
    H: int = 256
    W: int = 192
    ncores: int = 8
    P: int = 128
    NSEG: int = 64
    FRONT: int = 256
    REAR: int = 512

    @property
    def BP(self):  # batches per core
        return self.B // self.ncores

    @property
    def R(self):  # heatmap rows per core
        return self.BP * self.C

    @property
    def HWm(self):
        return self.H * self.W

    @property
    def SEGW(self):
        return self.HWm // self.NSEG

    @property
    def RPT(self):  # rows per tile-column
        return self.P // self.NSEG

    @property
    def NT(self):  # tile-columns per core
        return self.R // self.RPT

    @property
    def MARG(self):
        return self.W + 2

    @property
    def WINW(self):
        return self.SEGW + 2 * self.MARG

    @property
    def SHN(self):
        return self.R * self.HWm

    @property
    def NPAD(self):
        return self.FRONT + self.SHN + self.REAR

    @property
    def NTA(self):  # tile-columns in group A (rows 0..127)
        return self.P // self.RPT

    @property
    def NTB(self):  # tile-columns in group B (rows 128..R-1)
        return self.NT - self.NTA

    @property
    def GB(self):  # rows in group B
        return self.R - self.P


CFG = Cfg()

# stream schedule: group B tile-columns first, then group A with a taper so
# the last reduce is tiny.  Each entry is (start_col, n_cols).
def _schedule(c: Cfg):
    sched = []
    # group B: NTB columns as MD2 chunks
    col = c.NTA
    while col < c.NT:
        md = min(2, c.NT - col)
        sched.append((col, md))
        col += md
    # group A: MD4 until 4 columns remain, then 2, 1, 1
    col = 0
    while col < c.NTA:
        rem = c.NTA - col
        if rem > 4:
            md = 4
        elif rem == 4:
            md = 2
        elif rem >= 2:
            md = min(2, rem - 1) if rem > 1 else 1
        else:
            md = 1
        sched.append((col, md))
        col += md
    return sched


def build_program(cfg: Cfg):
    c = cfg
    assert c.P % c.NSEG == 0 and c.R % c.RPT == 0 and c.HWm % c.NSEG == 0
    assert c.FRONT >= c.MARG and c.REAR >= c.MARG
    sched = _schedule(c)
    assert sum(m for _, m in sched) == c.NT
    assert sorted(q for s, m in sched for q in range(s, s + m)) == list(range(c.NT))

    nc = bacc.Bacc(
        "TRN2", target_bir_lowering=False, debug=False, num_devices=c.ncores
    )
    xh = nc.dram_tensor("x", [c.NPAD], F32, kind="ExternalInput").ap()
    rbh = nc.dram_tensor("rowbase", [c.NT, c.RPT], F32, kind="ExternalInput").ap()
    io64h = nc.dram_tensor("iorev64", [c.NT, c.NSEG], F32, kind="ExternalInput").ap()
    io768h = nc.dram_tensor("iorev768", [c.P, c.SEGW], F32, kind="ExternalInput").ap()
    oh = nc.dram_tensor("out", [c.R, 3], F32, kind="ExternalOutput").ap()

    with ExitStack() as ctx:
        tc = ctx.enter_context(tile.TileContext(nc))
        xpool = ctx.enter_context(tc.tile_pool(name="xp", bufs=6))
        sp = ctx.enter_context(tc.tile_pool(name="sp", bufs=1))
        pp = ctx.enter_context(tc.tile_pool(name="pp", bufs=1, space="PSUM"))

        MA = sp.tile([c.P, c.NTA], F32, tag="MA")
        MB = sp.tile([c.P, c.NTB], F32, tag="MB")

        # constants
        ident = sp.tile([c.P, c.P], F32, tag="ident")
        make_identity(nc, ident[:])
        rbtA = sp.tile([c.NTA, c.RPT], F32, tag="rbtA")
        rbtB = sp.tile([c.NTB, c.RPT], F32, tag="rbtB")
        io64 = sp.tile([c.NT, c.NSEG], F32, tag="io64")
        io768 = sp.tile([c.P, c.SEGW], F32, tag="io768")

        # ---- phase 1: stream + per-(row, segment) maxima ---------------------
        def issue_dma(i, col, md):
            xt = xpool.tile([c.P, md * c.SEGW], F32, tag=f"xt{md}")
            off = c.FRONT + col * c.RPT * c.HWm
            src = bass.AP(
                xh.tensor,
                off,
                [
                    [c.HWm, c.RPT],
                    [c.SEGW, c.NSEG],
                    [c.RPT * c.HWm, md],
                    [1, c.SEGW],
                ],
            )
            eng = nc.sync if i % 2 == 0 else nc.scalar
            eng.dma_start(
                out=xt[:].rearrange("p (m u) -> p m u", m=md), in_=src
            )
            return xt

        def reduce_tile(xt, col, md):
            M, base = (MA, 0) if col < c.NTA else (MB, c.NTA)
            nc.vector.reduce_max(
                out=M[:, col - base : col - base + md],
                in_=xt[:].rearrange("p (m u) -> p m u", m=md),
                axis=AX.X,
            )

        # issue the B DMAs + constants first, then all A DMAs
        nb = c.NTB // 2  # number of B dma chunks
        tiles = {}
        for i, (col, md) in enumerate(sched[:nb]):
            tiles[col] = issue_dma(i, col, md)
        nc.sync.dma_start(out=rbtA[:], in_=rbh[0 : c.NTA])
        nc.sync.dma_start(out=rbtB[:], in_=rbh[c.NTA : c.NT])
        nc.scalar.dma_start(out=io64[:], in_=io64h[:])
        nc.scalar.dma_start(out=io768[:], in_=io768h[:])

        # ---- phase 2 helpers -------------------------------------------------
        def pregather(MT, gp, nt, rbt_s, tagp, cast_eng):
            """winner segment + window start for one group.

            MT: [nt, P] transposed maxima (MT[t, j*NSEG+s]); returns
            (P4 [nt, RPT, 3] packed (w0, score, sb), w0u [gp,1] u32 offsets,
             RT [gp, 3] row-major relayout tile)
            """
            MT3 = MT.rearrange("p (j s) -> p j s", j=c.RPT)
            P4 = sp.tile([nt, c.RPT * 3], F32, tag=f"P4{tagp}")
            P43 = P4[:].rearrange("p (j e) -> p j e", e=3)
            nc.vector.tensor_reduce(
                out=P43[:, :, 1:2], in_=MT3, axis=AX.X, op=OP.max
            )
            # srev = max((M == score) * iorev64) per (t, j); first segment on
            # ties (matches argmax), then sb = (NSEG-1-srev) * SEGW
            srev = sp.tile([nt, c.RPT], F32, tag=f"ss{tagp}")
            for j in range(c.RPT):
                mk = sp.tile([nt, c.NSEG], F32, tag=f"mk{tagp}{j}")
                scr = sp.tile([nt, c.NSEG], F32, tag=f"scr{tagp}{j}")
                nc.vector.tensor_tensor(
                    out=mk[:],
                    in0=MT[:, j * c.NSEG : (j + 1) * c.NSEG],
                    in1=P4[:, 3 * j + 1 : 3 * j + 2].to_broadcast([nt, c.NSEG]),
                    op=OP.is_equal,
                )
                nc.vector.tensor_tensor(
                    out=scr[:], in0=mk[:], in1=io64[0:nt], op=OP.mult
                )
                nc.vector.tensor_reduce(
                    out=srev[:, j : j + 1], in_=scr[:], axis=AX.X, op=OP.max
                )
            nc.vector.tensor_scalar(
                out=P43[:, :, 2:3],
                in0=srev[:, :, None],
                scalar1=-float(c.SEGW),
                scalar2=float((c.NSEG - 1) * c.SEGW),
                op0=OP.mult,
                op1=OP.add,
            )
            nc.vector.tensor_tensor(
                out=P43[:, :, 0:1],
                in0=P43[:, :, 2:3],
                in1=rbt_s[:, :, None],
                op=OP.add,
            )
            return P43

        def relayout(P43, gp, tagp):
            RT = sp.tile([gp, 3], F32, tag=f"RT{tagp}")
            nc.sync.dma_start(out=RT[:], in_=P43)
            return RT

        def cast_w0(RT, gp, tagp):
            w0u = sp.tile([gp, 1], U32, tag=f"w0u{tagp}")
            nc.vector.tensor_copy(out=w0u[:], in_=RT[:, 0:1])
            return w0u

        def gather_win(gp, w0u, tagp):
            win = sp.tile([gp, c.WINW], F32, tag=f"win{tagp}")
            nc.gpsimd.indirect_dma_start(
                out=win[:],
                out_offset=None,
                in_=xh[:, None],
                in_offset=bass.IndirectOffsetOnAxis(ap=w0u[:, 0:1], axis=0),
            )
            return win

        def postgather(win, RT, gp, tagp, div_eng, aux_eng):
            """masked-diff refinement + coordinate math for one group."""
            M0 = c.MARG
            mid = win[:, M0 : M0 + c.SEGW]
            diff = sp.tile([gp, 2 * c.SEGW], F32, tag=f"df{tagp}")
            aux_eng.tensor_tensor(
                out=diff[:, 0 : c.SEGW],
                in0=win[:, M0 + 1 : M0 + 1 + c.SEGW],
                in1=win[:, M0 - 1 : M0 - 1 + c.SEGW],
                op=OP.subtract,
            )
            aux_eng.tensor_tensor(
                out=diff[:, c.SEGW : 2 * c.SEGW],
                in0=win[:, M0 + c.W : M0 + c.W + c.SEGW],
                in1=win[:, M0 - c.W : M0 - c.W + c.SEGW],
                op=OP.subtract,
            )
            scr = sp.tile([gp, c.SEGW], F32, tag=f"pscr{tagp}")
            mkw = sp.tile([gp, c.SEGW], F32, tag=f"mkw{tagp}")
            ii = sp.tile([gp, 1], F32, tag=f"ii{tagp}")
            rstar = sp.tile([gp, 1], F32, tag=f"rs{tagp}")
            D = sp.tile([gp, 2], F32, tag=f"D{tagp}")
            score = RT[:, 1:2]
            # rstar = max((win == score) * iorev): first peak position on ties
            nc.vector.tensor_tensor(
                out=mkw[:], in0=mid,
                in1=score.to_broadcast([gp, c.SEGW]), op=OP.is_equal,
            )
            nc.vector.tensor_tensor(
                out=scr[:], in0=mkw[:], in1=io768[0:gp], op=OP.mult
            )
            nc.vector.tensor_reduce(
                out=rstar[:], in_=scr[:], axis=AX.X, op=OP.max
            )
            # one-hot select the +-1 / +-W differences at that exact position
            nc.vector.scalar_tensor_tensor(
                out=scr[:], in0=io768[0:gp], scalar=rstar[:],
                in1=diff[:, 0 : c.SEGW],
                op0=OP.is_equal, op1=OP.mult, accum_out=D[:, 0:1],
            )
            nc.vector.scalar_tensor_tensor(
                out=scr[:], in0=io768[0:gp], scalar=rstar[:],
                in1=diff[:, c.SEGW :],
                op0=OP.is_equal, op1=OP.mult, accum_out=D[:, 1:2],
            )

            # flat index within row; px, py via exact f32 division fixup
            O = sp.tile([gp, 3], F32, tag=f"O{tagp}")
            e = div_eng
            e.tensor_scalar(
                out=ii[:], in0=rstar[:], scalar1=-1.0,
                scalar2=float(c.SEGW - 1), op0=OP.mult, op1=OP.add,
            )
            idxm = sp.tile([gp, 1], F32, tag=f"idxm{tagp}")
            e.tensor_tensor(out=idxm[:], in0=RT[:, 2:3], in1=ii[:], op=OP.add)
            t1 = sp.tile([gp, 1], F32, tag=f"t1{tagp}")
            t2 = sp.tile([gp, 1], F32, tag=f"t2{tagp}")
            qi = sp.tile([gp, 1], I32, tag=f"qi{tagp}")
            e.tensor_scalar(
                out=t1[:], in0=idxm[:], scalar1=1.0 / c.W, scalar2=0.0013,
                op0=OP.mult, op1=OP.add,
            )
            e.tensor_copy(out=qi[:], in_=t1[:])
            e.tensor_copy(out=t1[:], in_=qi[:])
            e.tensor_scalar(
                out=t2[:], in0=t1[:], scalar1=-float(c.W), scalar2=None,
                op0=OP.mult,
            )
            e.tensor_tensor(out=t2[:], in0=idxm[:], in1=t2[:], op=OP.add)
            lo = sp.tile([gp, 1], F32, tag=f"lo{tagp}")
            e.tensor_scalar(
                out=lo[:], in0=t2[:], scalar1=0.0, scalar2=None, op0=OP.is_lt
            )
            e.tensor_tensor(out=t1[:], in0=t1[:], in1=lo[:], op=OP.subtract)
            e.tensor_scalar(
                out=lo[:], in0=t2[:], scalar1=float(c.W), scalar2=None,
                op0=OP.is_ge,
            )
            e.tensor_tensor(out=O[:, 1:2], in0=t1[:], in1=lo[:], op=OP.add)
            e.tensor_scalar(
                out=t2[:], in0=O[:, 1:2], scalar1=-float(c.W), scalar2=None,
                op0=OP.mult,
            )
            e.tensor_tensor(out=O[:, 0:1], in0=idxm[:], in1=t2[:], op=OP.add)
            mk = sp.tile([gp, 1], F32, tag=f"mk{tagp}")
            e.tensor_scalar(
                out=mk[:], in0=score, scalar1=0.0, scalar2=None, op0=OP.is_gt
            )
            e.tensor_tensor(
                out=O[:, 0:2], in0=O[:, 0:2],
                in1=mk[:].to_broadcast([gp, 2]), op=OP.mult,
            )
            # interior = (0 < px < W-1) & (0 < py < H-1)
            hi = sp.tile([gp, 2], F32, tag=f"hi{tagp}")
            nc.vector.memset(hi[:, 0:1], float(c.W - 1))
            nc.vector.memset(hi[:, 1:2], float(c.H - 1))
            ilo = sp.tile([gp, 2], F32, tag=f"ilo{tagp}")
            e.tensor_scalar(
                out=ilo[:], in0=O[:, 0:2], scalar1=0.0, scalar2=None,
                op0=OP.is_gt,
            )
            ihi = sp.tile([gp, 2], F32, tag=f"ihi{tagp}")
            e.tensor_tensor(out=ihi[:], in0=O[:, 0:2], in1=hi[:], op=OP.is_lt)
            e.tensor_tensor(out=ilo[:], in0=ilo[:], in1=ihi[:], op=OP.mult)
            intr = sp.tile([gp, 1], F32, tag=f"intr{tagp}")
            nc.vector.tensor_reduce(
                out=intr[:], in_=ilo[:], axis=AX.X, op=OP.min
            )
            # dx, dy = 0.25 * sign(D) * interior
            DG = sp.tile([gp, 2], F32, tag=f"DG{tagp}")
            DL = sp.tile([gp, 2], F32, tag=f"DL{tagp}")
            e.tensor_scalar(
                out=DG[:], in0=D[:], scalar1=0.0, scalar2=0.25,
                op0=OP.is_gt, op1=OP.mult,
            )
            e.tensor_scalar(
                out=DL[:], in0=D[:], scalar1=0.0, scalar2=0.25,
                op0=OP.is_lt, op1=OP.mult,
            )
            e.tensor_tensor(out=DG[:], in0=DG[:], in1=DL[:], op=OP.subtract)
            e.tensor_tensor(
                out=DG[:], in0=DG[:], in1=intr[:].to_broadcast([gp, 2]),
                op=OP.mult,
            )
            e.tensor_tensor(out=O[:, 0:2], in0=O[:, 0:2], in1=DG[:], op=OP.add)
            e.tensor_copy(out=O[:, 2:3], in_=score)
            return O

        # ---- group B (rows P..R-1): reduce + winner select, emitted before the
        # A-stream issues so B's relayout can sit mid-way in sync's queue ------
        for col, md in sched[:nb]:
            reduce_tile(tiles[col], col, md)
        mtpB = pp.tile([c.NTB, c.P], F32, tag="mtpB")
        nc.tensor.transpose(out=mtpB[:], in_=MB[:], identity=ident[:])
        MTB = sp.tile([c.NTB, c.P], F32, tag="MTB")
        nc.vector.tensor_copy(out=MTB[:], in_=mtpB[:])
        P43B = pregather(MTB[:], c.GB, c.NTB, rbtB[:], "b", nc.vector)

        # ---- A stream issues; B's tiny relayout after the 4th sync issue (by
        # then its inputs are long done, so sync never stalls on it) ----------
        RTB = None
        for i, (col, md) in enumerate(sched[nb:]):
            tiles[col] = issue_dma(nb + i, col, md)
            if i == 6:
                RTB = relayout(P43B, c.GB, "b")
        assert RTB is not None

        # ---- A reduces, with B's gather + post-gather staged in -------------
        for k, (col, md) in enumerate(sched[nb:]):
            reduce_tile(tiles[col], col, md)
            if k == 1:
                w0uB = cast_w0(RTB, c.GB, "b")
                winB = gather_win(c.GB, w0uB, "b")
            if k == 4:
                OB = postgather(winB[:], RTB[:], c.GB, "b", nc.vector, nc.gpsimd)
        nc.scalar.dma_start(out=oh[c.P : c.R], in_=OB[:])

        # ---- group A chain ---------------------------------------------------
        mtpA = pp.tile([c.NTA, c.P], F32, tag="mtpA")
        nc.tensor.transpose(out=mtpA[:], in_=MA[:], identity=ident[:])
        MTA = sp.tile([c.NTA, c.P], F32, tag="MTA")
        nc.vector.tensor_copy(out=MTA[:], in_=mtpA[:])
        P43A = pregather(MTA[:], c.P, c.NTA, rbtA[:], "a", nc.vector)
        RTA = relayout(P43A, c.P, "a")
        w0uA = cast_w0(RTA, c.P, "a")
        winA = gather_win(c.P, w0uA, "a")
        OA = postgather(winA[:], RTA[:], c.P, "a", nc.vector, nc.gpsimd)
        nc.sync.dma_start(out=oh[0 : c.P], in_=OA[:])

    nc.compile()
    return nc


def host_constants(cfg: Cfg):
    c = cfg
    r = np.arange(c.R, dtype=np.float64)
    rowbase = (c.FRONT + r * c.HWm - c.MARG).astype(np.float32).reshape(c.NT, c.RPT)
    iorev64 = np.tile(
        (c.NSEG - 1 - np.arange(c.NSEG)).astype(np.float32), (c.NT, 1)
    )
    iorev768 = np.tile(
        (c.SEGW - 1 - np.arange(c.SEGW)).astype(np.float32), (c.P, 1)
    )
    return rowbase, iorev64, iorev768


def shard_inputs(cfg: Cfg, x: np.ndarray):
    c = cfg
    rowbase, iorev64, iorev768 = host_constants(c)
    in_maps = []
    for k in range(c.ncores):
        shard = np.ascontiguousarray(
            x[k * c.BP : (k + 1) * c.BP], dtype=np.float32
        ).reshape(-1)
        xp = np.zeros(c.NPAD, np.float32)
        xp[c.FRONT : c.FRONT + c.SHN] = shard
        in_maps.append(
            {"x": xp, "rowbase": rowbase, "iorev64": iorev64, "iorev768": iorev768}
        )
    return in_maps


def assemble_out(cfg: Cfg, per_core_outs):
    c = cfg
    outs = [o.reshape(c.BP, c.C, 3).astype(np.float32) for o in per_core_outs]
    return np.concatenate(outs, axis=0)


_PROGRAM = None


def _program():
    global _PROGRAM
    if _PROGRAM is None:
        _PROGRAM = build_program(CFG)
    return _PROGRAM


def kernel(x: np.ndarray) -> np.ndarray:
    from concourse.bass_utils import run_bass_kernel_spmd

    c = CFG
    assert x.shape == (c.B, c.C, c.H, c.W), x.shape
    nc = _program()
    in_maps = shard_inputs(c, np.asarray(x))
    res = run_bass_kernel_spmd(nc, in_maps, core_ids=list(range(c.ncores)))
    return assemble_out(c, [res.results[k]["out"] for k in range(c.ncores)])
